# revision 36
# baseline (speedup 1.0000x reference)
"""DC_CE_Marginal_loss for Trainium2 — 8-core data-parallel Bass kernel, v2.

Shards the [B,C,D,H,W] volume along D across 8 NeuronCores, two launches:

  Launch A (counts): per-(b,c) voxel counts from the one-hot target.
      t is shipped as fp8 (0/1 exact); the whole reduction runs on the
      otherwise-idle TensorEngine as ones-vector matmuls accumulating in
      PSUM (sequential per-plane groups in one bank), then one ACT
      copy+accum -> [16,1] -> DRAM. No DVE streaming work at all.

  Launch B (main loss): compiled specialized on the per-sample present
      counts (n0, n1) derived from launch A. The host permutes channels
      present-first, so absent channels are skipped everywhere and no
      mask tensors are needed. All big elementwise ops are bf16 with
      flat innermost-contiguous APs so the DVE runs in 2x_1p mode:
        bg merge (absent logits -> class 0), wide exp (ACT), pairwise
        S-tree, fast reciprocal, q = e*r via a stride-0 broadcast TT,
        tq = t*q, u-term via fused tensor_tensor_reduce.
      Per-class seg/intersect reductions run on the TensorEngine as
      ones-matmuls into PSUM ([16,FCH] per chunk, sequential groups),
      accumulated into an SBUF [32, MMN] tile; ln(S+pad) is batched at
      the end on ACT (one table set with exp). Host sums the partial
      columns and finishes the loss.
"""
import os
import numpy as np
import ml_dtypes

B, C, D, H, W = 2, 8, 64, 160, 160
NCORES = 8
DS = D // NCORES            # depth slices per core
PLANE = DS * H * W          # voxels per (b,c) plane per core = 204800
P = 128
FREE = PLANE // P           # 1600
NCH = 4                     # chunks per sample plane
FCH = FREE // NCH           # 400
NVOX = B * D * H * W

NCOL = 3 * B * NCH          # lse / lnS / ln(ql) col per (b, chunk)

U_TTR = os.environ.get("K_U_TTR", "0") == "1"  # TTR dies on HW (runtime INTERNAL)
Q_BCAST = os.environ.get("K_Q_BCAST", "1") == "1"

_CACHE = {}


def _build_a():
    import concourse.bacc as bacc
    import concourse.tile as tile
    from concourse import mybir

    FA = mybir.ActivationFunctionType
    AL = mybir.AluOpType
    f32 = mybir.dt.float32
    f8 = mybir.dt.float8e4

    nc = bacc.Bacc("TRN2", num_devices=NCORES, name="loss_counts_v4")
    # [P, B*C, FREE]: one 25.6KB contiguous DMA row per partition
    t8 = nc.dram_tensor("t8", [P, B * C, FREE], f8, kind="ExternalInput")
    out = nc.dram_tensor("cnt", [P, B * C], f32, kind="ExternalOutput")

    # per-plane free-dim sums via accum_out, planes split DVE/ACT;
    # host sums the partition axis
    NSEG = 4
    with tile.TileContext(nc) as tc:
        with (
            tc.tile_pool(name="tin", bufs=3) as tin,
            tc.tile_pool(name="cpool", bufs=1) as cpool,
            tc.tile_pool(name="junk", bufs=2) as junkp,
        ):
            cnt = cpool.tile([P, B * C], f32)
            PLSEG = B * C // NSEG
            for seg in range(NSEG):
                t_sb = tin.tile([P, PLSEG, FREE], f8, tag="t")
                nc.sync.dma_start(
                    t_sb[:], t8[:, seg * PLSEG : (seg + 1) * PLSEG, :])
                for p in range(PLSEG):
                    bc = seg * PLSEG + p
                    jk = junkp.tile([P, FREE], f8, tag=f"j{bc % 2}")
                    if bc % 2 == 0:
                        nc.vector.tensor_scalar(
                            out=jk[:], in0=t_sb[:, p, :], scalar1=1.0,
                            scalar2=0.0, op0=AL.mult, op1=AL.add,
                            accum_out=cnt[:, bc : bc + 1])
                    else:
                        nc.scalar.activation(
                            out=jk[:], in_=t_sb[:, p, :], func=FA.Copy,
                            accum_out=cnt[:, bc : bc + 1])
            nc.sync.dma_start(out[:], cnt[:])
    nc.compile()
    return nc


import contextlib


@contextlib.contextmanager
def _pin_act_set(name="natural_log_exp_and_others"):
    """Make `name` the only table set offering Exp/Ln, so the table-load
    pass picks it for both -> one ACT_TABLE_LOAD, exp/ln interleave freely.
    Set *indices* are preserved (they are the act_func_set_id)."""
    import concourse.bacc as bacc
    from concourse import mybir
    orig = bacc.get_activation_tables
    FA = mybir.ActivationFunctionType
    strip = {FA.Exp, FA.Ln}

    def patched(arch):
        t = orig(arch)
        assert name in t and strip <= set(t[name]), (name, t.get(name))
        return {
            k: (v if k == name else set(v) - strip) for k, v in t.items()
        }

    bacc.get_activation_tables = patched
    try:
        yield
    finally:
        bacc.get_activation_tables = orig


def _build_b(n_present):
    """n_present: tuple of per-sample present-class counts (after the host
    permuted channels present-first)."""
    import concourse.bacc as bacc
    import concourse.tile as tile
    from concourse import mybir
    from concourse.bass import broadcast_tensor_aps

    FA = mybir.ActivationFunctionType
    AL = mybir.AluOpType
    f32, bf16 = mybir.dt.float32, mybir.dt.bfloat16

    L = max(n_present)

    nc = bacc.Bacc("TRN2", num_devices=NCORES, name="loss_main_v5")
    x = nc.dram_tensor("x", [B, NCH, P, C, FCH], bf16, kind="ExternalInput")
    t = nc.dram_tensor("t", [B, NCH, P, C, FCH], bf16, kind="ExternalInput")
    cols = nc.dram_tensor("cols", [P, NCOL], f32, kind="ExternalOutput")
    # segi: partial sums (host sums f axis);
    # kind 0 = seg, 1 = intersect, 2 = counts
    segi = nc.dram_tensor("segi", [1, 3, B, 8, FCH], f32,
                          kind="ExternalOutput")

    ctx_pin = _pin_act_set()
    ctx_pin.__enter__()
    with tile.TileContext(nc) as tc:
        with (
            tc.tile_pool(name="xin", bufs=3) as xin,
            tc.tile_pool(name="tin", bufs=3) as tin,
            tc.tile_pool(name="ework", bufs=2) as ework,
            tc.tile_pool(name="qwork", bufs=2) as qwork,
            tc.tile_pool(name="tqwork", bufs=2) as tqwork,
            tc.tile_pool(name="junkp", bufs=2) as junkp,
            tc.tile_pool(name="small", bufs=2) as small,
            tc.tile_pool(name="cpool", bufs=1) as cpool,
            tc.psum_pool(name="pp", bufs=1) as pp,
        ):
            colsb = cpool.tile([P, NCOL], f32)
            nc.vector.memset(colsb[:], 0.0)
            ones = cpool.tile([P, 1], bf16)
            nc.vector.memset(ones[:], 1.0)
            # persistent psum accumulators: region (kind, c) at partition
            # row 32*kind, bank c, cols [512c : 512c+FCH]; one accumulation
            # group per region per sample, drained by ACT after each sample
            ps = pp.tile([P, 8 * 512], f32)
            stage = cpool.tile([1, 3, B, 8, FCH], f32)
            padc = cpool.tile([P, B], f32)
            for b in range(B):
                nc.vector.memset(padc[:, b : b + 1], float(L - n_present[b]))
            NCC = B * NCH

            for b in range(B):
                n = n_present[b]
                for ch in range(NCH):
                    x_sb = xin.tile([P, C, FCH], bf16, tag="x")
                    nc.sync.dma_start(x_sb[:], x[b, ch])
                    t_sb = tin.tile([P, n, FCH], bf16, tag="t")
                    nc.sync.dma_start(t_sb[:], t[b, ch, :, 0:n, :])

                    # ---- bg merge: absent logits folded into class 0 ----
                    if n < C:
                        na = C - n
                        if na == 1:
                            bgs = x_sb[:, n, :]
                        else:
                            bg = small.tile([P, FCH], bf16, tag="bg")
                            nc.vector.tensor_tensor(
                                out=bg[:], in0=x_sb[:, n, :],
                                in1=x_sb[:, n + 1, :], op=AL.add)
                            for a in range(n + 2, C):
                                bg2 = small.tile([P, FCH], bf16, tag="bg")
                                nc.vector.tensor_tensor(
                                    out=bg2[:], in0=bg[:],
                                    in1=x_sb[:, a, :], op=AL.add)
                                bg = bg2
                            bgs = bg[:]
                        nc.vector.tensor_tensor(
                            out=x_sb[:, 0, :], in0=x_sb[:, 0, :],
                            in1=bgs, op=AL.add)

                    # ---- e = exp(x) over present channels (one wide op) ----
                    e_sb = ework.tile([P, n, FCH], bf16, tag="e")
                    nc.scalar.activation(
                        out=e_sb[:], in_=x_sb[:, 0:n, :], func=FA.Exp)

                    # ---- S = sum_c e_c (pairwise tree, bf16, flat) ----
                    S_t = small.tile([P, FCH], f32, tag="S")
                    S = S_t[:]
                    if n == 8:
                        s4 = small.tile([P, 4, FCH], bf16, tag="s4")
                        nc.vector.tensor_tensor(
                            out=s4[:], in0=e_sb[:, 0:4, :],
                            in1=e_sb[:, 4:8, :], op=AL.add)
                        s2 = small.tile([P, 2, FCH], bf16, tag="s2")
                        nc.vector.tensor_tensor(
                            out=s2[:], in0=s4[:, 0:2, :],
                            in1=s4[:, 2:4, :], op=AL.add)
                        nc.vector.tensor_tensor(
                            out=S, in0=s2[:, 0, :], in1=s2[:, 1, :],
                            op=AL.add)
                    elif n == 5:
                        s2 = small.tile([P, 2, FCH], bf16, tag="s2")
                        nc.vector.tensor_tensor(
                            out=s2[:], in0=e_sb[:, 0:2, :],
                            in1=e_sb[:, 2:4, :], op=AL.add)
                        s1 = small.tile([P, FCH], bf16, tag="s1")
                        nc.vector.tensor_tensor(
                            out=s1[:], in0=s2[:, 0, :], in1=s2[:, 1, :],
                            op=AL.add)
                        nc.vector.tensor_tensor(
                            out=S, in0=s1[:], in1=e_sb[:, 4, :], op=AL.add)
                    else:
                        # generic pairwise tree
                        cur = [e_sb[:, c, :] for c in range(n)]
                        lvl = 0
                        while len(cur) > 1:
                            nxt = []
                            for i in range(0, len(cur) - 1, 2):
                                if len(cur) == 2:
                                    o = S
                                else:
                                    ot = small.tile(
                                        [P, FCH], bf16, tag=f"st{lvl}{i}")
                                    o = ot[:]
                                nc.vector.tensor_tensor(
                                    out=o, in0=cur[i], in1=cur[i + 1],
                                    op=AL.add)
                                nxt.append(o)
                            if len(cur) % 2:
                                nxt.append(cur[-1])
                            cur = nxt
                            lvl += 1

                    # ---- r = 1/S: fast-approx recip, bf16 writeback ----
                    from concourse.dve_ops import (
                        RECIP_APPROX_FAST_CONSTS, RECIPROCAL_APPROX_FAST)
                    r16 = small.tile([P, 1, FCH], bf16, tag="r16")
                    rc = RECIP_APPROX_FAST_CONSTS
                    nc.vector._custom_dve(
                        RECIPROCAL_APPROX_FAST, out=r16[:, 0, :], in0=S,
                        s0=rc["s0"], s1=rc["s1"], imm2=rc["imm2"])

                    # ---- q = e * r (wide TT, r broadcast along C) ----
                    q_sb = qwork.tile([P, n, FCH], bf16, tag="q")
                    if Q_BCAST:
                        _, rb = broadcast_tensor_aps(e_sb[:], r16[:])
                        nc.vector.tensor_tensor(
                            out=q_sb[:], in0=e_sb[:], in1=rb, op=AL.mult)
                    else:
                        for c in range(n):
                            nc.vector.tensor_tensor(
                                out=q_sb[:, c, :], in0=e_sb[:, c, :],
                                in1=r16[:, 0, :], op=AL.mult)

                    # ---- tq = t * q ----
                    tq_sb = tqwork.tile([P, n, FCH], bf16, tag="tq")
                    nc.vector.tensor_tensor(
                        out=tq_sb[:], in0=t_sb[:], in1=q_sb[:], op=AL.mult)

                    # ---- ql = sum_c tq_c = q_label (one-hot: exact) ----
                    # u-term: sum(t*m) = sum ln(ql) + sum ln(S), on ACT below
                    ql_t = small.tile([P, FCH], bf16, tag="ql")
                    ql = ql_t[:]
                    if n == 8:
                        u4 = small.tile([P, 4, FCH], bf16, tag="u4")
                        nc.vector.tensor_tensor(
                            out=u4[:], in0=tq_sb[:, 0:4, :],
                            in1=tq_sb[:, 4:8, :], op=AL.add)
                        u2 = small.tile([P, 2, FCH], bf16, tag="u2")
                        nc.vector.tensor_tensor(
                            out=u2[:], in0=u4[:, 0:2, :],
                            in1=u4[:, 2:4, :], op=AL.add)
                        nc.vector.tensor_tensor(
                            out=ql, in0=u2[:, 0, :], in1=u2[:, 1, :],
                            op=AL.add)
                    else:
                        u2 = small.tile([P, 2, FCH], bf16, tag="u2")
                        nc.vector.tensor_tensor(
                            out=u2[:], in0=tq_sb[:, 0:2, :],
                            in1=tq_sb[:, 2:4, :], op=AL.add)
                        u1 = small.tile([P, FCH], bf16, tag="u1")
                        nc.vector.tensor_tensor(
                            out=u1[:], in0=u2[:, 0, :], in1=u2[:, 1, :],
                            op=AL.add)
                        nc.vector.tensor_tensor(
                            out=ql, in0=u1[:], in1=tq_sb[:, n - 1, :],
                            op=AL.add)

                    # ---- CE pieces on ACT (exp+ln share the pinned set):
                    # lse = sum ln(S+pad); u-term parts sum ln(S), sum ln(ql)
                    idx = b * NCH + ch
                    jln = small.tile([P, FCH], f32, tag="jln")
                    nc.scalar.activation(
                        out=jln[:], in_=S, func=FA.Ln,
                        bias=padc[:, b : b + 1], scale=1.0,
                        accum_out=colsb[:, idx : idx + 1])
                    if n < L:
                        nc.scalar.activation(
                            out=jln[:], in_=S, func=FA.Ln,
                            accum_out=colsb[:, NCC + idx : NCC + idx + 1])
                    nc.scalar.activation(
                        out=jln[:], in_=ql, func=FA.Ln,
                        accum_out=colsb[:, 2 * NCC + idx : 2 * NCC + idx + 1])

                    # ---- per-class reductions on TensorE: ones-stationary
                    # streaming, out [1, FCH] accumulated in the (kind, c)
                    # psum region across this sample's chunks. In the last
                    # chunk, go class-major and drain each class right after
                    # its final matmul (ACT/DVE split) so drains overlap the
                    # remaining matmuls instead of serializing at the end.
                    kinds = ((0, q_sb), (1, tq_sb), (2, t_sb))
                    if ch < NCH - 1:
                        for kind, src in kinds:
                            row = 32 * kind
                            for c in range(n):
                                nc.tensor.matmul(
                                    ps[row : row + 1,
                                       512 * c : 512 * c + FCH],
                                    ones[:],
                                    src[:, c, :],
                                    start=(ch == 0),
                                    stop=False,
                                    skip_group_check=True,
                                )
                    else:
                        for c in range(n):
                            for kind, src in kinds:
                                nc.tensor.matmul(
                                    ps[32 * kind : 32 * kind + 1,
                                       512 * c : 512 * c + FCH],
                                    ones[:],
                                    src[:, c, :],
                                    start=False,
                                    stop=True,
                                    skip_group_check=True,
                                )
                            for kind in range(3):
                                reg = ps[32 * kind : 32 * kind + 1,
                                         512 * c : 512 * c + FCH]
                                dst = stage[:, kind, b, c, :]
                                if kind == 1:
                                    nc.vector.tensor_scalar_mul(
                                        dst, reg, 1.0)
                                else:
                                    nc.scalar.copy(dst, reg)
                        if c == n - 1:
                            for kind in range(3):
                                nc.sync.dma_start(
                                    segi[:, kind, b, 0:n, :],
                                    stage[:, kind, b, 0:n, :])

            nc.sync.dma_start(cols[:], colsb[:])
    nc.compile()
    ctx_pin.__exit__(None, None, None)
    return nc


def _get(key, builder, *args):
    if key not in _CACHE:
        _CACHE[key] = builder(*args)
    return _CACHE[key]


def _run(nc, in_maps, out_names):
    if os.environ.get("K_SIM", "0") == "1":
        import concourse.bass_interp as bass_interp
        sim = bass_interp.MultiCoreSim(nc, NCORES)
        for k in range(NCORES):
            for name, arr in in_maps[k].items():
                sim.cores[k].tensor(name)[:] = arr
        sim.simulate()
        return [{o: sim.cores[k].tensor(o).copy() for o in out_names}
                for k in range(NCORES)]
    from concourse.bass_utils import run_bass_kernel_spmd
    return run_bass_kernel_spmd(
        nc, in_maps, core_ids=list(range(NCORES))).results


def run_a(t8maps):
    nc = _get("a", _build_a)
    results = _run(nc, [{"t8": tk} for tk in t8maps], ["cnt"])
    cnt_g = np.zeros((B, C), dtype=np.float64)
    for r in results:
        cnt_g += r["cnt"].astype(np.float64).sum(axis=0).reshape(B, C)
    return cnt_g


def run_b(xmaps, tmaps, n_present):
    nc = _get(("b", tuple(n_present)), _build_b, tuple(n_present))
    in_maps = [{"x": xmaps[k], "t": tmaps[k]} for k in range(NCORES)]
    results = _run(nc, in_maps, ["cols", "segi"])
    cols = np.zeros((NCOL,), dtype=np.float64)
    segs = np.zeros((B, 3, 8), dtype=np.float64)
    for r in results:
        cols += r["cols"].astype(np.float64).sum(axis=0)
        v = r["segi"].astype(np.float64).reshape(3, B, 8, FCH).sum(axis=3)
        segs += v.transpose(1, 0, 2)
    return cols, segs


def kernel(net_output, target):
    xs = np.asarray(net_output)
    ts = np.asarray(target)

    # class presence (layout/specialization decision only — the counts used
    # in the loss numerics are computed on-device in launch B)
    present = ts.reshape(B, C, -1).any(axis=2)
    perms = []
    n_present = []
    for b in range(B):
        pr = np.nonzero(present[b])[0]
        ab = np.nonzero(~present[b])[0]
        perms.append(np.concatenate([pr, ab]).astype(np.int64))
        n_present.append(int(len(pr)))
    n_present = tuple(n_present)
    L = max(n_present)

    # -------- launch B inputs: permuted present-first, chunked bf16 --------
    x6 = xs.reshape(B, C, NCORES, P, NCH, FCH)
    t6 = ts.reshape(B, C, NCORES, P, NCH, FCH)
    xp = np.stack([x6[b, perms[b]] for b in range(B)])  # [B,C,K,P,NCH,FCH]
    tp = np.stack([t6[b, perms[b]] for b in range(B)])
    # -> [K, B, NCH, P, C, FCH]
    xp = np.ascontiguousarray(
        xp.transpose(2, 0, 4, 3, 1, 5)).astype(ml_dtypes.bfloat16)
    tp = np.ascontiguousarray(
        tp.transpose(2, 0, 4, 3, 1, 5)).astype(ml_dtypes.bfloat16)
    xmaps = [xp[k] for k in range(NCORES)]
    tmaps = [tp[k] for k in range(NCORES)]

    cols, segs = run_b(xmaps, tmaps, n_present)
    # device-computed per-(b, permuted-class) voxel counts
    cnt_dev = segs[:, 2, :]

    # -------- host finish --------
    NCC = B * NCH
    lse_cols, lnS_cols, lnql_cols = cols[:NCC], cols[NCC:2 * NCC], cols[2 * NCC:]
    lse_sum = lse_cols.sum()
    u_sum = lnql_cols.sum()
    for b in range(B):
        if n_present[b] < L:
            u_sum += lnS_cols[b * NCH : (b + 1) * NCH].sum()
        else:  # pad == 0: ln(S+pad) == ln(S), reuse the lse columns
            u_sum += lse_cols[b * NCH : (b + 1) * NCH].sum()
    ce = (lse_sum - u_sum) / NVOX

    dice_is = []
    for b in range(B):
        n = n_present[b]
        seg = segs[b, 0, :n]
        inter = segs[b, 1, :n]
        cnt = cnt_dev[b, :n]
        dice_c = 2.0 * inter / (cnt + seg + 1e-5)
        dice_is.append(1.0 - dice_c.sum() / n)
    dc = np.mean(dice_is)
    return np.asarray(0.5 * ce + 0.5 * dc, dtype=np.float32)
